# revision 1
# baseline (speedup 1.0000x reference)
"""MLA/GQA attention kernel for Trainium2, 8-core SPMD.

Sharding: 16 units = (4 batch) x (2 kv-head pairs); each core gets one
batch element + 2 KV heads (+ their 8 Q heads).  W_q/W_k/W_v column-
sharded, W_o row-sharded; the two partial outputs per batch element are
summed on the host at gather time (no device collectives).

Host-side folds (no nonlinearity in between, so exact up to fp32
rounding): k_latent/v_latent restore matrices are folded into W_k/W_v.

Per-core pipeline (all matmuls float32r: full PE rate at free>=256,
~1.4e-4 rel err):
  P: xT resident in SBUF; project qT (via DRAM round-trip), kT, v.
  A: flash-style S^T = kT' qT per 128-key chunk; exp on ScalarE;
     ctx^T += v' expS; den += ones' expS (all-ones 128x128 lhsT gives
     the softmax denominator replicated across partitions); normalize
     ctx^T by reciprocal(den) on VectorE.
  O: out = ctx^T' @ W_o row-block, accumulate 8 head chunks in PSUM.
"""
import sys

sys.path.insert(0, "/opt/trn_rl_repo")

import numpy as np

import concourse.bass as bass  # noqa: F401  (registers rust bindings)
import concourse.mybir as mybir
import concourse.tile as tile
from concourse import bacc, bass_utils

D = 2048
T = 2048
NH = 16          # query heads total
NKV = 4          # kv heads total
DH = 128
LAT = 64
B = 4
NCORE = 8
HQ = 8           # q heads per core
HKV = 2          # kv heads per core
SCALE = 1.0 / np.sqrt(np.float32(DH))

F32 = mybir.dt.float32
F32R = mybir.dt.float32r
EXP = mybir.ActivationFunctionType.Exp

_CACHE = {}


def _build(reps=1, wide_exp=False, s_bufs=2, exp_bufs=4, q_bufs=4, cd_bufs=2):
    nc = bacc.Bacc("TRN2", target_bir_lowering=False, debug=False)
    xt_d = nc.dram_tensor("xt", [D, T], F32R, kind="ExternalInput").ap()
    wq_d = nc.dram_tensor("wq", [D, HQ * DH], F32R, kind="ExternalInput").ap()
    wk_d = nc.dram_tensor("wk", [D, HKV * DH], F32R, kind="ExternalInput").ap()
    wv_d = nc.dram_tensor("wv", [D, HKV * DH], F32R, kind="ExternalInput").ap()
    wo_d = nc.dram_tensor("wo", [HQ * DH, D], F32R, kind="ExternalInput").ap()
    out_d = nc.dram_tensor("out", [T, D], F32, kind="ExternalOutput").ap()

    NCC = D // 128   # 16 contraction chunks
    NT = T // 128    # 16 token chunks
    NQ = T // 512    # 4 wide token tiles

    with tile.TileContext(nc) as tc:
      for rep in range(reps):
        R = f"r{rep}"
        with tc.tile_pool(name=f"persist{R}", bufs=1) as persist, \
             tc.tile_pool(name=f"dram{R}", bufs=1, space="DRAM") as dram:
            qt_dram = dram.tile([HQ, DH, T], F32R, name=f"qtd{R}")
            ones_f = persist.tile([128, 128], F32, name=f"onesf{R}")
            nc.vector.memset(ones_f[:], 1.0)
            ones = persist.tile([128, 128], F32R, name=f"ones{R}")
            nc.vector.tensor_copy(ones[:], ones_f[:])
            kts = [persist.tile([DH, T], F32R, tag=f"kt{g}{R}", name=f"kt{g}{R}") for g in range(HKV)]
            vts = [persist.tile([128, HKV * DH], F32R, tag=f"vt{t}{R}", name=f"vt{t}{R}")
                   for t in range(NT)]
            # ---------------- Phase P: projections ----------------
            with tc.tile_pool(name=f"px{R}", bufs=1) as px, \
                 tc.tile_pool(name=f"pw{R}", bufs=3) as pw, \
                 tc.tile_pool(name=f"pst{R}", bufs=3) as pst:
                xts, wvs = [], []
                for cc in range(NCC):
                    xtile = px.tile([128, T], F32R, tag=f"x{cc}", name=f"x{cc}{R}")
                    nc.sync.dma_start(xtile[:], xt_d[cc * 128:(cc + 1) * 128, :])
                    xts.append(xtile)
                    wvt = px.tile([128, HKV * DH], F32R, tag=f"wv{cc}", name=f"wvs{cc}{R}")
                    nc.sync.dma_start(wvt[:], wv_d[cc * 128:(cc + 1) * 128, :])
                    wvs.append(wvt)

                # qT: one head at a time -> 4 psum banks, double-buffered
                # across heads so evictions overlap the next head's matmuls
                with tc.tile_pool(name=f"ppq{R}", bufs=2, space="PSUM") as ppq:
                  for hs in range(HQ):
                      psq = [ppq.tile([128, 512], F32, tag=f"psq{i}{R}", name=f"psq{hs}_{i}{R}")
                             for i in range(NQ)]
                      for cc in range(NCC):
                          wqc = pw.tile([128, 128], F32R, tag="wq")
                          nc.sync.dma_start(
                              wqc[:], wq_d[cc * 128:(cc + 1) * 128,
                                           hs * 128:(hs + 1) * 128])
                          for qt in range(NQ):
                              nc.tensor.matmul(
                                  psq[qt][:],
                                  wqc[:],
                                  xts[cc][:, qt * 512:(qt + 1) * 512],
                                  start=(cc == 0), stop=(cc == NCC - 1))
                      for qt in range(NQ):
                          st = pst.tile([128, 512], F32R, tag="qst")
                          nc.scalar.copy(st[:], psq[qt][:])
                          nc.sync.dma_start(
                              qt_dram[hs, :, qt * 512:(qt + 1) * 512], st[:])

                # kT: both kv heads at once (2 x 4 = 8 psum banks)
                pps_cm = tc.tile_pool(name=f"pps{R}", bufs=1, space="PSUM")
                pps = pps_cm.__enter__()
                psk = [pps.tile([128, 512], F32, tag=f"ps{i}{R}", name=f"psk{i}{R}") for i in range(8)]
                for cc in range(NCC):
                    wkc = pw.tile([128, HKV * DH], F32R, tag="wk")
                    nc.sync.dma_start(wkc[:], wk_d[cc * 128:(cc + 1) * 128, :])
                    for g in range(HKV):
                        for qt in range(NQ):
                            nc.tensor.matmul(
                                psk[g * 4 + qt][:],
                                wkc[:, g * 128:(g + 1) * 128],
                                xts[cc][:, qt * 512:(qt + 1) * 512],
                                start=(cc == 0), stop=(cc == NCC - 1))
                for g in range(HKV):
                    for qt in range(NQ):
                        nc.scalar.copy(kts[g][:, qt * 512:(qt + 1) * 512],
                                       psk[g * 4 + qt][:])

                # v in natural [token, dv] orientation, 8 chunks at a time
                for th in range(2):
                    psv = [pps.tile([128, HKV * DH], F32, tag=f"ps{i}{R}", name=f"psv{i}{R}")
                           for i in range(8)]
                    for cc in range(NCC):
                        for tl in range(8):
                            tg = th * 8 + tl
                            nc.tensor.matmul(
                                psv[tl][:],
                                xts[cc][:, tg * 128:(tg + 1) * 128],
                                wvs[cc][:],
                                start=(cc == 0), stop=(cc == NCC - 1))
                    for tl in range(8):
                        nc.scalar.copy(vts[th * 8 + tl][:], psv[tl][:])
                pps_cm.__exit__(None, None, None)

            # ---------------- Phases A+O share the ctx pool ----------------
            with tc.tile_pool(name=f"actx{R}", bufs=1) as actx:
              ctxts = [actx.tile([DH, T], F32R, tag=f"ctx{h}{R}", name=f"ctx{h}{R}")
                       for h in range(HQ)]
              # -------------- Phase A: attention --------------
              with tc.tile_pool(name=f"aq{R}", bufs=q_bufs) as aq, \
                 tc.tile_pool(name=f"aexp{R}", bufs=exp_bufs) as aexp, \
                 tc.tile_pool(name=f"arec{R}", bufs=2) as arec, \
                 tc.tile_pool(name=f"asps{R}", bufs=s_bufs, space="PSUM") as asps, \
                 tc.tile_pool(name=f"aps{R}", bufs=cd_bufs, space="PSUM") as aps:
                for h in range(HQ):
                    g = h // 4
                    for qc in range(NQ):
                        qtile = aq.tile([DH, 512], F32R, tag="q")
                        nc.sync.dma_start(
                            qtile[:], qt_dram[h, :, qc * 512:(qc + 1) * 512])
                        ps_ctx = aps.tile([128, 512], F32, tag="ps_ctx")
                        ps_den = aps.tile([128, 512], F32, tag="ps_den")
                        if not wide_exp:
                          for kc in range(NT):
                            ps_n = asps.tile([128, 512], F32, tag="ps_s")
                            nc.tensor.matmul(
                                ps_n[:], kts[g][:, kc * 128:(kc + 1) * 128],
                                qtile[:], start=True, stop=True)
                            exn = aexp.tile([128, 512], F32R, tag="exp")
                            nc.scalar.activation(exn[:], ps_n[:], EXP,
                                                 scale=float(SCALE))
                            nc.tensor.matmul(
                                ps_ctx[:],
                                vts[kc][:, g * 128:(g + 1) * 128], exn[:],
                                start=(kc == 0), stop=(kc == NT - 1))
                            nc.tensor.matmul(
                                ps_den[:], ones[:], exn[:],
                                start=(kc == 0), stop=(kc == NT - 1))
                        else:
                         for kc2 in range(NT // 2):
                             # two key chunks per PSUM tile so the exp
                             # activation runs 1024 wide (amortizes the ~352
                             # cycle ACT fixed cost; ACT was the attention
                             # bottleneck at 512)
                             ps_s = asps.tile([128, 1024], F32, tag="ps_s")
                             ex = aexp.tile([128, 1024], F32R, tag="exp")
                             for j in range(2):
                                 kc = 2 * kc2 + j
                                 nc.tensor.matmul(
                                     ps_s[:, j * 512:(j + 1) * 512],
                                     kts[g][:, kc * 128:(kc + 1) * 128],
                                     qtile[:], start=True, stop=True)
                             nc.scalar.activation(ex[:], ps_s[:], EXP,
                                                  scale=float(SCALE))
                             for j in range(2):
                                 kc = 2 * kc2 + j
                                 exj = ex[:, j * 512:(j + 1) * 512]
                                 nc.tensor.matmul(
                                     ps_ctx[:],
                                     vts[kc][:, g * 128:(g + 1) * 128], exj,
                                     start=(kc == 0), stop=(kc == NT - 1))
                                 nc.tensor.matmul(
                                     ps_den[:], ones[:], exj,
                                     start=(kc == 0), stop=(kc == NT - 1))
                        rec = arec.tile([128, 512], F32, tag="rec")
                        nc.vector.reciprocal(rec[:], ps_den[:])
                        nc.vector.tensor_mul(
                            ctxts[h][:, qc * 512:(qc + 1) * 512],
                            ps_ctx[:], rec[:])

              # ---------------- Phase O: output projection ----------------
              with tc.tile_pool(name=f"ow{R}", bufs=1) as ow, \
                 tc.tile_pool(name=f"ost{R}", bufs=3) as ost, \
                 tc.tile_pool(name=f"ops{R}", bufs=2, space="PSUM") as ops:
                wos = []
                for hc in range(HQ):
                    wot = ow.tile([128, D], F32R, tag=f"wo{hc}", name=f"wo{hc}{R}")
                    nc.sync.dma_start(wot[:], wo_d[hc * 128:(hc + 1) * 128, :])
                    wos.append(wot)
                for tg in range(NT):
                    pso = ops.tile([128, D], F32, tag="pso")
                    for od in range(4):
                        for hc in range(HQ):
                            nc.tensor.matmul(
                                pso[:, od * 512:(od + 1) * 512],
                                ctxts[hc][:, tg * 128:(tg + 1) * 128],
                                wos[hc][:, od * 512:(od + 1) * 512],
                                start=(hc == 0), stop=(hc == HQ - 1))
                    st = ost.tile([128, D], F32, tag="ostage")
                    nc.any.tensor_copy(st[:], pso[:])
                    nc.sync.dma_start(out_d[tg * 128:(tg + 1) * 128, :], st[:])

    nc.compile()
    return nc


LAST_RESULTS = None


def kernel(x, W_q, W_k, W_v, W_k_to_latent, W_v_to_latent,
           W_k_from_latent, W_v_from_latent, W_o):
    global LAST_RESULTS
    x = np.asarray(x, np.float32)
    # Fold latent compress->restore into W_k / W_v (linear, so exact).
    mk = np.asarray(W_k_to_latent, np.float32) @ np.asarray(W_k_from_latent, np.float32)
    mv = np.asarray(W_v_to_latent, np.float32) @ np.asarray(W_v_from_latent, np.float32)
    wk_eff = np.einsum("dgh,hk->dgk", np.asarray(W_k, np.float32).reshape(D, NKV, DH),
                       mk).reshape(D, NKV * DH)
    wv_eff = np.einsum("dgh,hk->dgk", np.asarray(W_v, np.float32).reshape(D, NKV, DH),
                       mv).reshape(D, NKV * DH)
    wq = np.asarray(W_q, np.float32)
    wo = np.asarray(W_o, np.float32)

    if "nc" not in _CACHE:
        _CACHE["nc"] = _build()
    nc = _CACHE["nc"]

    in_maps = []
    for c in range(NCORE):
        b, p = c // 2, c % 2
        in_maps.append({
            "xt": np.ascontiguousarray(x[b].T),
            "wq": np.ascontiguousarray(wq[:, p * 1024:(p + 1) * 1024]),
            "wk": np.ascontiguousarray(wk_eff[:, p * 256:(p + 1) * 256]),
            "wv": np.ascontiguousarray(wv_eff[:, p * 256:(p + 1) * 256]),
            "wo": np.ascontiguousarray(wo[p * 1024:(p + 1) * 1024, :]),
        })
    res = bass_utils.run_bass_kernel_spmd(nc, in_maps, core_ids=list(range(NCORE)))
    LAST_RESULTS = res
    out = np.empty((B, T, D), np.float32)
    for b in range(B):
        out[b] = res.results[2 * b]["out"] + res.results[2 * b + 1]["out"]
    return out

